# revision 7
# baseline (speedup 1.0000x reference)
"""CQVAE loss kernel for Trainium2, data-parallel over batch on 8 NeuronCores.

loss = kld(qy) + mse(gather(rzs), zs[:, :Sg]) + bias(best, best_gt)
       + bias(gather(pts), gts)
where bias(p, g) = mse(p, g) + 10 * mse(p[..., MARK, :], g[..., MARK, :]).

Each core handles 16 of the 128 batches.  The host stages two fused bf16
tensors per core: `gath` = [rzs | pts | pad] rows (indexed by mapping) and
`seq` = [zs' | gts | pad] rows (the pair targets), both 1280 columns.  The
device gathers `gath` rows via dma_gather (SWDGE ucode, two queues so both
Q7 SWDGE cores emit descriptors in parallel; the mlp ucode library is
preloaded up front), subtracts the `seq` tiles on the vector engine, then
squares + partial-sums on the scalar/vector engines into a [128, 20] stats
tile; the host folds partitions and cores.
"""

import sys

import numpy as np

try:
    import concourse  # noqa: F401
except ImportError:  # pragma: no cover
    sys.path.insert(0, "/opt/trn_rl_repo")

import ml_dtypes

import concourse.bass as bass
import concourse.mybir as mybir
import concourse.tile as tile
from concourse import bacc, library_config
from concourse.bass_utils import run_bass_kernel_spmd

F32 = mybir.dt.float32
BF16 = mybir.dt.bfloat16
I16 = mybir.dt.int16
AX = mybir.AxisListType
OP = mybir.AluOpType
ACTF = mybir.ActivationFunctionType

NCORES = 8
B, S, SG, D, P, V = 128, 256, 128, 1024, 118, 64
BL = B // NCORES  # batches per core
P2 = 2 * P  # 236 floats per point-row
C = D + P2  # fused row payload: [rzs|pts] or [zs|gts]
CP = 1280  # padded row width (dma_gather needs elem_size*2 % 256 == 0)
MARK = (0, 29, 88, 117)
EPS = 1e-20
ALPHA = 10.0

KG = 2  # batches per gather group
NG = BL // KG  # groups
AE_ON_V = (6, 7)  # groups whose ae square runs on the vector engine
NIND = 3  # leading groups gathered via native indirect DMA (no ucode lib needed)
NSTAT = 20
# stats columns: 0..7 bias_sq per group; 8..15 ae_sq per group;
#                16 best_sq; 17 best_mark_sq; 18 kld_num; 19 bias_mark_sq
AE0 = 8

QN = BL * S // 128  # qy rows per partition
QCOLS = QN * V

_module = None
last_results = None  # BassKernelResults of the most recent run (for profiling)


def _build_module():
    nc = bacc.Bacc(num_swdge_queues=2)

    gath = nc.dram_tensor("gath", [BL * S, CP], BF16, kind="ExternalInput")
    seq = nc.dram_tensor("seq", [NG * 128, KG * CP], BF16, kind="ExternalInput")
    qy = nc.dram_tensor("qy", [128, QCOLS], BF16, kind="ExternalInput")
    bb = nc.dram_tensor("bb", [BL, 2 * P2], BF16, kind="ExternalInput")
    # idx16[16c + r, j] = flat gath row of gathered position k = 16j + r
    # (k = b*128 + i), replicated over the 8 Q7 cores c
    idx = nc.dram_tensor("idx", [128, BL * SG // 16], I16, kind="ExternalInput")
    idx32 = nc.dram_tensor("idx32", [SG, BL], mybir.dt.int32, kind="ExternalInput")
    out = nc.dram_tensor("out", [128, NSTAT], F32, kind="ExternalOutput")

    ICOL = KG * SG // 16  # idx columns per group

    with tile.TileContext(nc) as tc:
        with (
            tc.tile_pool(name="gt", bufs=1) as gtp,
            tc.tile_pool(name="sm", bufs=1) as sm,
            tc.tile_pool(name="cst", bufs=1) as cst,
        ):
            # start the Q7 ucode library fetch before anything else
            nc.gpsimd.load_library(library_config.mlp)

            idx_t = cst.tile([128, BL * SG // 16], I16)
            nc.sync.dma_start(idx_t[:], idx[:])
            idx32_t = cst.tile([SG, BL], mybir.dt.int32)
            nc.sync.dma_start(idx32_t[:], idx32[:])

            stats = cst.tile([128, NSTAT], F32)
            nc.vector.memset(stats[:], 0.0)
            bm = cst.tile([128, NG * len(MARK)], F32)  # per-group mark sums

            gts_, sts = [], []
            for g in range(NG):
                gt = gtp.tile([128, KG * CP], BF16, tag=f"gt{g}")
                if g < NIND:
                    with tc.high_priority():
                        for k in range(KG):
                            b = g * KG + k
                            nc.gpsimd.indirect_dma_start(
                                out=gt[:, k * CP : (k + 1) * CP],
                                out_offset=None,
                                in_=gath[:],
                                in_offset=bass.IndirectOffsetOnAxis(
                                    ap=idx32_t[:, b : b + 1], axis=0
                                ),
                            )
                else:
                    nc.gpsimd.dma_gather(
                        out_ap=gt[:].rearrange("p (k c) -> p k c", c=CP),
                        in_ap=gath[:],
                        idxs_ap=idx_t[:, g * ICOL : (g + 1) * ICOL],
                        num_idxs=KG * SG,
                        num_idxs_reg=KG * SG,
                        elem_size=CP,
                        queue_num=g % 2,
                    )
                st = sm.tile([128, KG * CP], BF16, tag=f"st{g}")
                (nc.sync if g % 2 == 0 else nc.scalar).dma_start(
                    st[:], seq[g * 128 : (g + 1) * 128, :]
                )
                gts_.append(gt)
                sts.append(st)
                if g == 0:
                    qy_t = cst.tile([128, QCOLS], BF16)
                    nc.scalar.dma_start(qy_t[:], qy[:])
                    bt = cst.tile([BL, 2 * P2], BF16)
                    nc.scalar.dma_start(bt[:], bb[:])

            # --- KLD: sum q * (log(q + eps) - log(1/V)) via log(V*q + V*eps)
            lg = cst.tile([128, QCOLS], BF16)
            ebias = cst.tile([128, 1], F32)
            nc.vector.memset(ebias[:], float(V) * EPS)
            nc.scalar.activation(lg[:], qy_t[:], ACTF.Ln, bias=ebias[:], scale=float(V))
            nc.vector.scalar_tensor_tensor(
                out=lg[:],
                in0=lg[:],
                scalar=0.0,
                in1=qy_t[:],
                op0=OP.subtract,
                op1=OP.mult,
                accum_out=stats[:, 18:19],
            )

            # --- BEST: [BL, 236|236] fused tile ---
            nc.vector.tensor_sub(bt[:, :P2], bt[:, :P2], bt[:, P2:])
            nc.scalar.activation(
                bt[:, :P2], bt[:, :P2], ACTF.Square, accum_out=stats[:BL, 16:17]
            )
            bm4 = cst.tile([BL, len(MARK)], F32)
            for j, m in enumerate(MARK):
                nc.vector.reduce_sum(
                    out=bm4[:, j : j + 1], in_=bt[:, 2 * m : 2 * m + 2], axis=AX.X
                )
            nc.vector.reduce_sum(out=stats[:BL, 17:18], in_=bm4[:], axis=AX.X)

            # --- per-group diff, squares & partial sums ---
            for g in range(NG):
                gt, st = gts_[g], sts[g]
                nc.vector.tensor_sub(gt[:], gt[:], st[:])
                gt3 = gt[:].rearrange("p (k c) -> p k c", c=CP)
                ae_out = stats[:, AE0 + g : AE0 + g + 1]
                if g in AE_ON_V:
                    nc.vector.scalar_tensor_tensor(
                        out=gt3[:, :, 0:D],
                        in0=gt3[:, :, 0:D],
                        scalar=0.0,
                        in1=gt3[:, :, 0:D],
                        op0=OP.subtract,
                        op1=OP.mult,
                        accum_out=ae_out,
                    )
                else:
                    nc.scalar.activation(
                        gt3[:, :, 0:D], gt3[:, :, 0:D], ACTF.Square, accum_out=ae_out
                    )
                nc.scalar.activation(
                    gt3[:, :, D:C],
                    gt3[:, :, D:C],
                    ACTF.Square,
                    accum_out=stats[:, g : g + 1],
                )
                for j, m in enumerate(MARK):
                    c0 = D + 2 * m
                    nc.vector.reduce_sum(
                        out=bm[:, g * len(MARK) + j : g * len(MARK) + j + 1],
                        in_=gt3[:, :, c0 : c0 + 2],
                        axis=AX.XY,
                    )

            nc.vector.reduce_sum(out=stats[:, 19:20], in_=bm[:], axis=AX.X)

            # ship per-partition stats; the host folds the 128 partitions
            nc.sync.dma_start(out[:], stats[:])

    nc.compile()
    return nc


def kernel(
    zs, rzs, pts, best, qy, gts, best_gt, mapping, vector_dims, **trace_kwargs
):
    global _module, last_results
    vd = int(np.asarray(vector_dims))
    assert vd == V, f"kernel compiled for vector_dims={V}, got {vd}"

    if _module is None:
        _module = _build_module()

    bf = ml_dtypes.bfloat16
    zs = np.asarray(zs, dtype=np.float32)
    rzs = np.asarray(rzs, dtype=np.float32)
    pts = np.asarray(pts, dtype=np.float32)
    gts = np.asarray(gts, dtype=np.float32)
    qy = np.asarray(qy, dtype=np.float32)
    mapping = np.asarray(mapping).astype(np.int32)
    best2 = np.asarray(best, dtype=np.float32).reshape(B, P2)
    bgt2 = np.asarray(best_gt, dtype=np.float32).reshape(B, P2)

    NI = BL * SG  # gathered rows per core
    base = (np.arange(BL, dtype=np.int32) * S)[:, None]
    in_maps = []
    for c in range(NCORES):
        sl = slice(c * BL, (c + 1) * BL)
        # gath rows (b, s): [rzs | pts | 0pad]
        gath = np.zeros((BL * S, CP), dtype=bf)
        gath[:, :D] = rzs[sl].reshape(BL * S, D)
        gath[:, D:C] = pts[sl].reshape(BL * S, P2)
        # seq rows (g, i, bg): [zs' | gts | 0pad]
        seq = np.zeros((BL, SG, CP), dtype=np.float32)
        seq[:, :, :D] = zs[sl][:, :SG]
        seq[:, :, D:C] = gts[sl].reshape(BL, SG, P2)
        seq = seq.reshape(NG, KG, SG, CP).transpose(0, 2, 1, 3)
        # gathered position k = b*128 + i -> idx16[k % 16, k // 16], x8 cores
        flat = (mapping[sl] + base).astype(np.int16).reshape(NI)  # k-order
        idx16 = np.zeros((16, NI // 16), np.int16)
        idx16[np.arange(NI) % 16, np.arange(NI) // 16] = flat
        in_maps.append(
            {
                "gath": gath,
                "seq": np.ascontiguousarray(seq.reshape(NG * 128, KG * CP)).astype(bf),
                "qy": qy[sl].reshape(128, QCOLS).astype(bf),
                "bb": np.concatenate([best2[sl], bgt2[sl]], axis=1).astype(bf),
                "idx": np.tile(idx16, (8, 1)),
                "idx32": np.ascontiguousarray((mapping[sl] + base).T.astype(np.int32)),
            }
        )

    last_results = run_bass_kernel_spmd(
        _module, in_maps, list(range(NCORES)), **trace_kwargs
    )
    parts = np.stack(
        [
            np.asarray(r["out"], dtype=np.float64).reshape(128, NSTAT).sum(axis=0)
            for r in last_results.results
        ]
    )
    tot = parts.sum(axis=0)

    ae_loss = tot[AE0 : AE0 + NG].sum() / (B * SG * D)
    bias_loss = tot[:NG].sum() / (B * SG * P2) + ALPHA * tot[19] / (
        B * SG * 2 * len(MARK)
    )
    kld_loss = tot[18] / (B * S)
    best_mse = tot[16] / (B * P2) + ALPHA * tot[17] / (B * 2 * len(MARK))

    return np.array(kld_loss + ae_loss + best_mse + bias_loss, dtype=np.float32)


# revision 9
# speedup vs baseline: 1.0614x; 1.0614x over previous
"""CQVAE loss kernel for Trainium2, data-parallel over batch on 8 NeuronCores.

loss = kld(qy) + mse(gather(rzs), zs[:, :Sg]) + bias(best, best_gt)
       + bias(gather(pts), gts)
where bias(p, g) = mse(p, g) + 10 * mse(p[..., MARK, :], g[..., MARK, :]).

Each core handles 16 of the 128 batches.  The host stages two fused bf16
tensors per core: `gath` = [rzs | pts | pad] rows (indexed by mapping) and
`seq` = [zs' | gts | pad] rows (the pair targets), both 1280 columns.  The
device gathers `gath` rows via dma_gather (SWDGE ucode, two queues so both
Q7 SWDGE cores emit descriptors in parallel; the mlp ucode library is
preloaded up front), subtracts the `seq` tiles on the vector engine, then
squares + partial-sums on the scalar/vector engines into a [128, 20] stats
tile; the host folds partitions and cores.
"""

import sys

import numpy as np

try:
    import concourse  # noqa: F401
except ImportError:  # pragma: no cover
    sys.path.insert(0, "/opt/trn_rl_repo")

import ml_dtypes

import concourse.bass as bass
import concourse.mybir as mybir
import concourse.tile as tile
from concourse import bacc, library_config
from concourse.bass_utils import run_bass_kernel_spmd

F32 = mybir.dt.float32
BF16 = mybir.dt.bfloat16
I16 = mybir.dt.int16
AX = mybir.AxisListType
OP = mybir.AluOpType
ACTF = mybir.ActivationFunctionType

NCORES = 8
B, S, SG, D, P, V = 128, 256, 128, 1024, 118, 64
BL = B // NCORES  # batches per core
P2 = 2 * P  # 236 floats per point-row
C = D + P2  # fused row payload: [rzs|pts] or [zs|gts]
CP = 1280  # padded row width (dma_gather needs elem_size*2 % 256 == 0)
MARK = (0, 29, 88, 117)
EPS = 1e-20
ALPHA = 10.0

KG = 2  # batches per gather group
NG = BL // KG  # groups
AE_ON_V = (5, 6, 7)  # groups whose ae square runs on the vector engine
NSTAT = 28
# stats columns: 0..7 bias_sq per group; 8..15 ae_sq per group;
#                16 best_sq; 17 best_mark_sq; 18 kld_num;
#                20..27 bias_mark_sq per group.  pts/gts (and best) point
#                columns are host-permuted so the 4 MARK pairs sit in the
#                last 8 bias columns -> marks reduce contiguously.
AE0 = 8
MK0 = 20
NMARK = 2 * len(MARK)

QN = BL * S // 128  # qy rows per partition
QCOLS = QN * V

_module = None
last_results = None  # BassKernelResults of the most recent run (for profiling)


def _build_module():
    nc = bacc.Bacc(num_swdge_queues=2)

    gath = nc.dram_tensor("gath", [BL * S, CP], BF16, kind="ExternalInput")
    seq = nc.dram_tensor("seq", [NG * 128, KG * CP], BF16, kind="ExternalInput")
    qy = nc.dram_tensor("qy", [128, QCOLS], BF16, kind="ExternalInput")
    bb = nc.dram_tensor("bb", [BL, 2 * P2], BF16, kind="ExternalInput")
    # idx16[16c + r, j] = flat gath row of gathered position k = 16j + r
    # (k = b*128 + i), replicated over the 8 Q7 cores c
    idx = nc.dram_tensor("idx", [128, BL * SG // 16], I16, kind="ExternalInput")
    out = nc.dram_tensor("out", [128, NSTAT], F32, kind="ExternalOutput")

    ICOL = KG * SG // 16  # idx columns per group

    with tile.TileContext(nc) as tc:
        with (
            tc.tile_pool(name="gt", bufs=1) as gtp,
            tc.tile_pool(name="sm", bufs=1) as sm,
            tc.tile_pool(name="cst", bufs=1) as cst,
        ):
            # start the Q7 ucode library fetch before anything else
            nc.gpsimd.load_library(library_config.mlp)

            idx_t = cst.tile([128, BL * SG // 16], I16)
            nc.sync.dma_start(idx_t[:], idx[:])

            stats = cst.tile([128, NSTAT], F32)
            nc.vector.memset(stats[:], 0.0)

            gts_, sts = [], []
            for g in range(NG):
                gt = gtp.tile([128, KG * CP], BF16, tag=f"gt{g}")
                nc.gpsimd.dma_gather(
                    out_ap=gt[:].rearrange("p (k c) -> p k c", c=CP),
                    in_ap=gath[:],
                    idxs_ap=idx_t[:, g * ICOL : (g + 1) * ICOL],
                    num_idxs=KG * SG,
                    num_idxs_reg=KG * SG,
                    elem_size=CP,
                    queue_num=g % 2,
                )
                st = sm.tile([128, KG * CP], BF16, tag=f"st{g}")
                (nc.sync if g % 2 == 0 else nc.scalar).dma_start(
                    st[:], seq[g * 128 : (g + 1) * 128, :]
                )
                gts_.append(gt)
                sts.append(st)
                if g == 0:
                    qy_t = cst.tile([128, QCOLS], BF16)
                    nc.scalar.dma_start(qy_t[:], qy[:])
                    bt = cst.tile([BL, 2 * P2], BF16)
                    nc.scalar.dma_start(bt[:], bb[:])

            # --- KLD: sum q * (log(q + eps) - log(1/V)) via log(V*q + V*eps)
            lg = cst.tile([128, QCOLS], BF16)
            ebias = cst.tile([128, 1], F32)
            nc.vector.memset(ebias[:], float(V) * EPS)
            nc.scalar.activation(lg[:], qy_t[:], ACTF.Ln, bias=ebias[:], scale=float(V))
            nc.vector.scalar_tensor_tensor(
                out=lg[:],
                in0=lg[:],
                scalar=0.0,
                in1=qy_t[:],
                op0=OP.subtract,
                op1=OP.mult,
                accum_out=stats[:, 18:19],
            )

            # --- BEST: [BL, 236|236] fused tile ---
            nc.vector.tensor_sub(bt[:, :P2], bt[:, :P2], bt[:, P2:])
            nc.scalar.activation(
                bt[:, :P2], bt[:, :P2], ACTF.Square, accum_out=stats[:BL, 16:17]
            )
            nc.vector.reduce_sum(
                out=stats[:BL, 17:18], in_=bt[:, P2 - NMARK : P2], axis=AX.X
            )

            # --- per-group diff, squares & partial sums ---
            for g in range(NG):
                gt, st = gts_[g], sts[g]
                nc.vector.tensor_sub(gt[:], gt[:], st[:])
                gt3 = gt[:].rearrange("p (k c) -> p k c", c=CP)
                ae_out = stats[:, AE0 + g : AE0 + g + 1]
                if g in AE_ON_V:
                    nc.vector.scalar_tensor_tensor(
                        out=gt3[:, :, 0:D],
                        in0=gt3[:, :, 0:D],
                        scalar=0.0,
                        in1=gt3[:, :, 0:D],
                        op0=OP.subtract,
                        op1=OP.mult,
                        accum_out=ae_out,
                    )
                else:
                    nc.scalar.activation(
                        gt3[:, :, 0:D], gt3[:, :, 0:D], ACTF.Square, accum_out=ae_out
                    )
                nc.scalar.activation(
                    gt3[:, :, D:C],
                    gt3[:, :, D:C],
                    ACTF.Square,
                    accum_out=stats[:, g : g + 1],
                )
                nc.vector.reduce_sum(
                    out=stats[:, MK0 + g : MK0 + g + 1],
                    in_=gt3[:, :, C - NMARK : C],
                    axis=AX.XY,
                )

            # ship per-partition stats; the host folds the 128 partitions
            nc.sync.dma_start(out[:], stats[:])

    nc.compile()
    return nc


def kernel(
    zs, rzs, pts, best, qy, gts, best_gt, mapping, vector_dims, **trace_kwargs
):
    global _module, last_results
    vd = int(np.asarray(vector_dims))
    assert vd == V, f"kernel compiled for vector_dims={V}, got {vd}"

    if _module is None:
        _module = _build_module()

    bf = ml_dtypes.bfloat16
    zs = np.asarray(zs, dtype=np.float32)
    rzs = np.asarray(rzs, dtype=np.float32)
    pts = np.asarray(pts, dtype=np.float32)
    gts = np.asarray(gts, dtype=np.float32)
    qy = np.asarray(qy, dtype=np.float32)
    mapping = np.asarray(mapping).astype(np.int32)
    perm = [p for p in range(P) if p not in MARK] + list(MARK)
    pts = np.ascontiguousarray(pts[:, :, perm, :])
    gts = np.ascontiguousarray(gts[:, :, perm, :])
    best2 = np.asarray(best, dtype=np.float32)[:, perm, :].reshape(B, P2)
    bgt2 = np.asarray(best_gt, dtype=np.float32)[:, perm, :].reshape(B, P2)

    NI = BL * SG  # gathered rows per core
    base = (np.arange(BL, dtype=np.int32) * S)[:, None]
    in_maps = []
    for c in range(NCORES):
        sl = slice(c * BL, (c + 1) * BL)
        # gath rows (b, s): [rzs | pts | 0pad]
        gath = np.zeros((BL * S, CP), dtype=bf)
        gath[:, :D] = rzs[sl].reshape(BL * S, D)
        gath[:, D:C] = pts[sl].reshape(BL * S, P2)
        # seq rows (g, i, bg): [zs' | gts | 0pad]
        seq = np.zeros((BL, SG, CP), dtype=np.float32)
        seq[:, :, :D] = zs[sl][:, :SG]
        seq[:, :, D:C] = gts[sl].reshape(BL, SG, P2)
        seq = seq.reshape(NG, KG, SG, CP).transpose(0, 2, 1, 3)
        # gathered position k = b*128 + i -> idx16[k % 16, k // 16], x8 cores
        flat = (mapping[sl] + base).astype(np.int16).reshape(NI)  # k-order
        idx16 = np.zeros((16, NI // 16), np.int16)
        idx16[np.arange(NI) % 16, np.arange(NI) // 16] = flat
        in_maps.append(
            {
                "gath": gath,
                "seq": np.ascontiguousarray(seq.reshape(NG * 128, KG * CP)).astype(bf),
                "qy": qy[sl].reshape(128, QCOLS).astype(bf),
                "bb": np.concatenate([best2[sl], bgt2[sl]], axis=1).astype(bf),
                "idx": np.tile(idx16, (8, 1)),
            }
        )

    last_results = run_bass_kernel_spmd(
        _module, in_maps, list(range(NCORES)), **trace_kwargs
    )
    parts = np.stack(
        [
            np.asarray(r["out"], dtype=np.float64).reshape(128, NSTAT).sum(axis=0)
            for r in last_results.results
        ]
    )
    tot = parts.sum(axis=0)

    ae_loss = tot[AE0 : AE0 + NG].sum() / (B * SG * D)
    bias_loss = tot[:NG].sum() / (B * SG * P2) + ALPHA * tot[MK0 : MK0 + NG].sum() / (
        B * SG * 2 * len(MARK)
    )
    kld_loss = tot[18] / (B * S)
    best_mse = tot[16] / (B * P2) + ALPHA * tot[17] / (B * 2 * len(MARK))

    return np.array(kld_loss + ae_loss + best_mse + bias_loss, dtype=np.float32)
